# revision 28
# baseline (speedup 1.0000x reference)
"""Trainium2 Bass kernel for the masked MQA attention block (nn_Attention_4252017623134).

Sharding: pure data-parallel over batch. b=8 batch elements, 8 NeuronCores,
one batch element per core, weights replicated. No collectives.

Per-core math (n=1024, d=1024, h=16, dh=64, inner=1024):
  context = x                      (pre-norm residual branch feeds K/V)
  xn  = layernorm(x) * g_in
  q   = xn @ Wq.T   (per head, scaled by 1/8 = dh^-0.5, folded into exp scale)
  k,v = context @ Wkv.T (single shared KV head) + null_kv token
  att = softmax(mask(q k^T / 8))   (padding + causal(key j visible iff j <= i))
  out = layernorm(att @ v @ Wo.T) * g_out

Key design decisions (v2 — fp8 DoubleRow projections):
  * All deep-K projections (q, kv, null-score) run as fp8e4m3 DoubleRow
    matmuls with an error-compensated hi/lo split: A@B ~ Ah@Bh + Ah@Bl +
    Al@Bh where Ah=fp8(A), Al=fp8(A-Ah). DoubleRow contracts 256 rows per
    instruction at 0.5 cycles/output-column, so the 3-term split costs
    0.75x of bf16 while injecting only ~1.2e-3 relative error (validated
    on HW). x is pre-scaled by 8 and the weights by 32 host-side so the
    fp8 residuals stay in e4m3's normal range; the 1/256 compensations
    fold into the host-computed rstd row (q/null paths) and the kv-evac
    scalar multiply.
  * LN1 folded into the q-projection: q_i = r_i * (Wq'' @ x_i) with
    Wq'' = Wq*diag(g) - outer(Wq@g, 1)/D precomputed on HOST and
    r_i = rsqrt(var_i+eps)/256 shipped as a row like the baseline.
  * Null-token scores via a TRANSPOSED projection: pnullT[i, h] accumulated
    with x as the stationary operand (16 moving columns -> ~0.8k PE cycles
    instead of 8k), exp'd as one [128, 8x16] activation, PE-transposed to
    row form, and spread over partitions 0/32/64/96 (unull_r4) so the PV
    null matmuls can address any head from a legal lhsT base partition.
  * Padding mask applied by ZEROING masked k/v columns; masked j gives
    u=exp(0)=1 but contributes v_j=0 and is excluded from the denominator
    via a mask column appended to V.
  * Scores stay bf16 (fp8 q/k injects ~5e-2 error - measured), computed
    transposed (simT[j,i], exact visible windows). Exp groups pack the 8
    j-tile windows into exactly 3x1536 columns ({0,4},{1,3},{2,5,6,7}),
    each a single [128,1536] 3-bank PSUM tile -> 3 activation instructions
    per head instead of 5 (the Act engine is the #2 engine at ~75us).
    Q-projection moved wholly to phase A to free the PSUM banks for this.
  * P@V runs NATURAL: lhsT = u[j, i-chunk], rhs = v_nat [j, 64ch + mask
    col]; softmax denominator lands in PSUM col 64; division fused into
    the PV evac as a per-partition reciprocal multiply. PV output returns
    to [c, i] via bf16 PE transposes staged through a 1-bank PSUM tile
    shared with the pv pool (ptr allocated after the 4th pv tile so pv
    stays double-buffered).
  * Engine choreography: input DMAs split across SP, Act-HWDGE and
    gpsimd-SWDGE queues (xh per-tile first so the kv projection starts
    ~1.5us in); causal-band zeroing (mtri) on DVE; LN2 gain multiply on
    gpsimd; dependency-free identity matmuls warm the PE p-state.
"""

import contextlib

import numpy as np
import ml_dtypes

import concourse.bass as bass
import concourse.bacc as bacc
import concourse.tile as tile
import concourse.mybir as mybir
from concourse.bass_utils import run_bass_kernel_spmd
from concourse.masks import make_identity

N = 1024          # sequence length per core
D = 1024          # model dim
H = 16            # query heads
DH = 64           # head dim
INNER = H * DH    # 1024
NT = N // 128     # 8 i-tiles / j-tiles / d-tiles
EPS = 1e-5
SW = 32.0         # host pre-scale on weights (fp8 range centering)
SX = 8.0          # host pre-scale on x
SINV = 1.0 / (SW * SX)

F32 = mybir.dt.float32
BF16 = mybir.dt.bfloat16
F8 = mybir.dt.float8e4
U8 = mybir.dt.uint8
AF = mybir.ActivationFunctionType
ALU = mybir.AluOpType
DR = mybir.MatmulPerfMode.DoubleRow

# exp groups per head: j-tile windows (N - 128t) packed into three
# [128, 1536] fp32 (3-bank) PSUM tiles; 1024+512, 896+640, 768+384+256+128.
EXP_GROUPS = [(0, 4), (1, 3), (2, 5, 6, 7)]
GW = 1536
U_OFF = {}
_off = 0
for _g in EXP_GROUPS:
    for _t in _g:
        U_OFF[_t] = _off
        _off += N - 128 * _t
U_COLS = _off  # 4608


def _bank_chunks(c0, c1):
    """Split [c0, c1) at 512-column (2KB fp32 PSUM bank) boundaries."""
    out = []
    while c0 < c1:
        nxt = min(c1, (c0 // 512 + 1) * 512)
        out.append((c0, nxt))
        c0 = nxt
    return out


def _emit(nc, apply_gout=True):
    # ---------------- DRAM I/O ----------------
    # all big operands ship pre-packed [128, NT, cols] (partition-major,
    # one contiguous descriptor per partition)
    xh_d = nc.dram_tensor("xh", [128, NT, N], F8, kind="ExternalInput")
    xl_d = nc.dram_tensor("xl", [128, NT, N], F8, kind="ExternalInput")
    # wq is packed PAIR-major: [128, pair, t, 128] so per-pair slices are one
    # contiguous descriptor per partition
    wqh_d = nc.dram_tensor("wqh", [128, NT, NT, 128], F8, kind="ExternalInput")
    wql_d = nc.dram_tensor("wql", [128, NT, NT, 128], F8, kind="ExternalInput")
    wkvh_d = nc.dram_tensor("wkvh", [128, NT, 2 * DH], F8, kind="ExternalInput")
    wkvl_d = nc.dram_tensor("wkvl", [128, NT, 2 * DH], F8, kind="ExternalInput")
    wnh_d = nc.dram_tensor("wnh", [128, NT, H], F8, kind="ExternalInput")
    wnl_d = nc.dram_tensor("wnl", [128, NT, H], F8, kind="ExternalInput")
    woh_d = nc.dram_tensor("woh", [128, NT, D], F8, kind="ExternalInput")
    wol_d = nc.dram_tensor("wol", [128, NT, D], F8, kind="ExternalInput")
    vnull_d = nc.dram_tensor("vnull", [DH], BF16, kind="ExternalInput")
    mask_d = nc.dram_tensor("mask", [N], U8, kind="ExternalInput")
    rrow_d = nc.dram_tensor("rrow", [N], F32, kind="ExternalInput")
    gout_d = nc.dram_tensor("gout", [D], F32, kind="ExternalInput")
    out_d = nc.dram_tensor("out", [N, D], F32, kind="ExternalOutput")

    d_ = dict(xh_d=xh_d, xl_d=xl_d, wqh_d=wqh_d, wql_d=wql_d,
              wkvh_d=wkvh_d, wkvl_d=wkvl_d, wnh_d=wnh_d, wnl_d=wnl_d,
              woh_d=woh_d, wol_d=wol_d, vnull_d=vnull_d, mask_d=mask_d,
              rrow_d=rrow_d,
              gout_d=gout_d, out_d=out_d)
    with tile.TileContext(nc) as tc:
        _emit_tile(nc, tc, d_, apply_gout)
    return nc


def _emit_tile(nc, tc, d_, apply_gout):
    xh_d, xl_d = d_["xh_d"], d_["xl_d"]
    wqh_d, wql_d = d_["wqh_d"], d_["wql_d"]
    wkvh_d, wkvl_d = d_["wkvh_d"], d_["wkvl_d"]
    wnh_d, wnl_d = d_["wnh_d"], d_["wnl_d"]
    woh_d, wol_d = d_["woh_d"], d_["wol_d"]
    vnull_d, mask_d = d_["vnull_d"], d_["mask_d"]
    rrow_d, gout_d, out_d = d_["rrow_d"], d_["gout_d"], d_["out_d"]

    ctx = contextlib.ExitStack()
    with ctx:
        consts = ctx.enter_context(tc.tile_pool(name="consts", bufs=1))
        persist = ctx.enter_context(tc.tile_pool(name="persist", bufs=1))
        stage = ctx.enter_context(tc.tile_pool(name="stage", bufs=6))

        # ---------------- constants (no DMA deps: emitted first so the PE
        # warm-up and gpsimd run from t=0) ----------------
        ident = consts.tile([128, 128], BF16)
        make_identity(nc, ident[:])
        # causal 0/1 band mask: keep u[j_rel, i_rel] iff i_rel >= j_rel
        mtri = consts.tile([128, 128], BF16)
        nc.gpsimd.memset(mtri[:], 1.0)
        nc.gpsimd.affine_select(out=mtri[:], in_=mtri[:], compare_op=ALU.is_ge,
                                fill=0.0, base=0, pattern=[[1, 128]],
                                channel_multiplier=-1)
        eps_t = consts.tile([128, 1], F32)
        nc.vector.memset(eps_t[:], EPS)

        # ------------- persistent tiles -------------
        kT2 = persist.tile([128, N], BF16, tag="kT2")       # k^T in both halves
        vts = persist.tile([128, N], BF16, tag="vts")   # rows 64:128 = masked v^T
        v_nat = persist.tile([128, NT, DH + 1], BF16, tag="v_nat")  # col 64 = mask
        vnull16 = persist.tile([128, DH + 1], BF16, tag="vnull16")  # rows 0/32/64
        unull = persist.tile([H, NT, 128], BF16, tag="unull")
        unull_r4 = persist.tile([128, 6 * N], BF16, tag="unull_r4")  # rows 0/32/64
        unullT = persist.tile([128, NT, H], BF16, tag="unullT")
        outT = persist.tile([128, NT, NT, 128], BF16, tag="outT")  # [c,pair,it,i]
        outT8h = persist.tile([128, NT, NT, 128], F8, tag="outT8h")
        outT8l = persist.tile([128, NT, NT, 128], F8, tag="outT8l")
        rbroad = persist.tile([128, N], F32, tag="rbroad")  # rstd/256 broadcast
        rcolT = persist.tile([128, NT], F32, tag="rcolT")   # rstd/256 column form
        qT = persist.tile([128, NT, N], BF16, tag="qT")     # q^T (pair slabs)
        xh = persist.tile([128, NT, N], F8, tag="xh")       # x^T hi (x8)
        xl = persist.tile([128, NT, N], F8, tag="xl")       # x^T lo residual
        wqh = persist.tile([128, NT, NT, 128], F8, tag="wqh")  # [p, pair, t, ch]
        wql = persist.tile([128, NT, NT, 128], F8, tag="wql")
        wkvh = persist.tile([128, NT, 2 * DH], F8, tag="wkvh")
        wkvl = persist.tile([128, NT, 2 * DH], F8, tag="wkvl")
        wnh = persist.tile([128, NT, H], F8, tag="wnh")
        wnl = persist.tile([128, NT, H], F8, tag="wnl")
        woh = persist.tile([128, NT, D], F8, tag="woh")
        wol = persist.tile([128, NT, D], F8, tag="wol")
        gout_b = persist.tile([128, D], F32, tag="gout_b")
        maskb_u8 = consts.tile([128, N], U8)
        maskc_u8 = consts.tile([128, NT], U8)
        maskb = consts.tile([128, N], BF16)
        maskc = consts.tile([128, NT], BF16)

        # ---- DMA issues. The DMA transfers serialize on ONE channel
        # (~0.385ns per per-partition byte), so channel ORDER sets the
        # phase-B start: kv/q0 inputs first, masks cheap-early, big late
        # tensors (wo, gout) last. Act queue kept to 5 issues (SEQ cost). ----
        nc.sync.dma_start(out=maskb_u8[:],
                          in_=bass.AP(tensor=mask_d, offset=0,
                                      ap=[[0, 128], [1, N]]))
        nc.scalar.dma_start(out=wkvh[:], in_=wkvh_d.ap())
        nc.sync.dma_start(out=xh[:, 0:4, :], in_=xh_d[:, 0:4, :])
        nc.scalar.dma_start(out=wkvl[:], in_=wkvl_d.ap())
        nc.sync.dma_start(out=xh[:, 4:8, :], in_=xh_d[:, 4:8, :])
        nc.scalar.dma_start(out=wqh[:, 0, :, :], in_=wqh_d[:, 0, :, :])
        nc.sync.dma_start(out=wql[:, 0, :, :], in_=wql_d[:, 0, :, :])
        nc.scalar.dma_start(out=wqh[:, 1, :, :], in_=wqh_d[:, 1, :, :])
        nc.sync.dma_start(out=rbroad[:],
                          in_=bass.AP(tensor=rrow_d, offset=0,
                                      ap=[[0, 128], [1, N]]))
        nc.scalar.dma_start(out=maskc_u8[:],
                            in_=bass.AP(tensor=mask_d, offset=0,
                                        ap=[[1, 128], [128, NT]]))
        nc.sync.dma_start(out=wql[:, 1, :, :], in_=wql_d[:, 1, :, :])
        nc.sync.dma_start(out=wnh[:], in_=wnh_d.ap())
        nc.sync.dma_start(out=wnl[:], in_=wnl_d.ap())
        nc.sync.dma_start(out=rcolT[:],
                          in_=bass.AP(tensor=rrow_d, offset=0,
                                      ap=[[1, 128], [128, NT]]))
        for m in range(2, NT):
            nc.sync.dma_start(out=wqh[:, m, :, :], in_=wqh_d[:, m, :, :])
            nc.sync.dma_start(out=wql[:, m, :, :], in_=wql_d[:, m, :, :])
        for r in (0, 32, 64):
            nc.sync.dma_start(out=vnull16[r:r + 1, 0:DH],
                              in_=bass.AP(tensor=vnull_d, offset=0,
                                          ap=[[0, 1], [1, DH]]))
        nc.sync.dma_start(out=gout_b[:],
                          in_=bass.AP(tensor=gout_d, offset=0,
                                      ap=[[0, 128], [1, D]]))
        nc.sync.dma_start(out=woh[:], in_=woh_d.ap())
        nc.sync.dma_start(out=wol[:], in_=wol_d.ap())

        # xl SWDGE issues first        # xl SWDGE issues first        # xl SWDGE issues first        # xl SWDGE issues first        # xl SWDGE issues first        # xl SWDGE issues first (no deps), then the mask converts (which
        # wait on the maskb DMA)
        nc.gpsimd.dma_start(out=xl[:, 0:4, :], in_=xl_d[:, 0:4, :])
        nc.gpsimd.dma_start(out=xl[:, 4:8, :], in_=xl_d[:, 4:8, :])
        nc.gpsimd.tensor_copy(maskb[:], maskb_u8[:])
        nc.gpsimd.tensor_copy(maskc[:], maskc_u8[:])
        nc.gpsimd.memset(vnull16[:, DH:DH + 1], 1.0)

        def q_pair(pool, m, tag="pvtr"):
            for ch in range(2):
                cs = slice(ch * 512, (ch + 1) * 512)
                pq = pool.tile([128, 512], F32, tag=tag, name="pq")
                i_ = 0
                for (a, b, tp) in [(wqh, xh, 0), (wqh, xh, 1),
                                   (wqh, xh, 2), (wqh, xh, 3),
                                   (wqh, xl, 0), (wqh, xl, 1),
                                   (wqh, xl, 2), (wqh, xl, 3),
                                   (wql, xh, 0), (wql, xh, 1),
                                   (wql, xh, 2), (wql, xh, 3)]:
                    nc.tensor.matmul(pq[:],
                                     a[:, m, 2 * tp:2 * tp + 2, :],
                                     b[:, 2 * tp:2 * tp + 2, cs],
                                     start=(i_ == 0), stop=(i_ == 11),
                                     perf_mode=DR)
                    i_ += 1
                nc.vector.scalar_tensor_tensor(
                    out=qT[:, m, cs], in0=pq[:], scalar=1.0,
                    in1=rbroad[:, cs], op0=ALU.mult, op1=ALU.mult)

        # ====== Phase A: projections (fp8 DoubleRow hi/lo), null scores ======
        with tc.tile_pool(name="psA", bufs=1, space="PSUM") as psA, \
             tc.tile_pool(name="psQ", bufs=2, space="PSUM") as psQp:
            pkv = psA.tile([128, N], F32, tag="pkv")
            pwarm = psA.tile([128, 128], F32, tag="pwarm")

            def warm(n):
                # PE p-state bridge: dependency-free matmuls fill DMA-wait
                # gaps so the ramp (2.4GHz after 3us continuous) survives
                for w_ in range(n):
                    nc.tensor.matmul(pwarm[:], ident[:], ident[:],
                                     start=(w_ == 0), stop=(w_ == n - 1))

            warm(34)

            # --- kv projection: 3-term hi/lo DoubleRow (tp order follows
            #     the split xh/xl DMA halves) ---
            for ch in range(2):
                cs = slice(ch * 512, (ch + 1) * 512)
                i_ = 0
                for (a, b, tp) in [(wkvh, xh, 0), (wkvh, xh, 1),
                                   (wkvl, xh, 0), (wkvl, xh, 1),
                                   (wkvh, xl, 0), (wkvh, xl, 1),
                                   (wkvh, xh, 2), (wkvh, xh, 3),
                                   (wkvl, xh, 2), (wkvl, xh, 3),
                                   (wkvh, xl, 2), (wkvh, xl, 3)]:
                    nc.tensor.matmul(pkv[:, cs], a[:, 2 * tp:2 * tp + 2, :],
                                     b[:, 2 * tp:2 * tp + 2, cs],
                                     start=(i_ == 0), stop=(i_ == 11),
                                     perf_mode=DR)
                    i_ += 1
            # evac: masked k^T (low half), masked v^T (rows 64:128)
            nc.vector.scalar_tensor_tensor(out=kT2[0:64, :], in0=pkv[0:64, :],
                                           scalar=SINV, in1=maskb[0:64, :],
                                           op0=ALU.mult, op1=ALU.mult)
            # k^T duplicate into partitions 64:128 for parity-B score matmuls
            nc.gpsimd.dma_start(out=kT2[64:128, :], in_=kT2[0:64, :])

            # --- q projection: 3-term hi/lo DoubleRow, LN1 via rstd row ---
            warm(14)
            q_pair(psQp, 0, tag="pq")
            # v evac after the q0 evacs (v is needed ~5us later than qT0)
            nc.vector.scalar_tensor_tensor(out=vts[64:128, :],
                                           in0=pkv[64:128, :],
                                           scalar=SINV, in1=maskb[64:128, :],
                                           op0=ALU.mult, op1=ALU.mult)

            # --- null-score projection, TRANSPOSED (x stationary) ---
            warm(12)
            pnullT = psA.tile([128, NT, H], F32, tag="pnullT")
            for it in range(NT):
                i_ = 0
                for (a, b) in ((xh, wnh), (xh, wnl), (xl, wnh)):
                    for tp in range(4):
                        nc.tensor.matmul(pnullT[:, it, :],
                                         a[:, 2 * tp:2 * tp + 2,
                                           it * 128:(it + 1) * 128],
                                         b[:, 2 * tp:2 * tp + 2, :],
                                         start=(i_ == 0), stop=(i_ == 11),
                                         perf_mode=DR)
                        i_ += 1
            # null exp straight from PSUM with the per-i rstd/2048 scale
            rcolT8 = stage.tile([128, NT], F32, tag="rcol8", bufs=1)
            nc.vector.tensor_scalar(out=rcolT8[:], in0=rcolT[:], scalar1=0.125,
                                    scalar2=None, op0=ALU.mult)
            for t in range(NT):
                nc.scalar.activation(out=unullT[:, t, :], in_=pnullT[:, t, :],
                                     func=AF.Exp, scale=rcolT8[:, t:t + 1])
            for t in range(NT):
                nc.gpsimd.tensor_copy(v_nat[:, t, DH:DH + 1], maskc[:, t:t + 1])
            # (q1 and the null/v transposes run inside phase B's first pair,
            #  off the phase-A PE critical path)

        # ============ Phase B: attention, per head pair ============
        # psS: 2 x [128,1536] (3 banks each); psB: 2 x 1 bank shared by the
        # q-projection pq, PV pv, and transpose-staging ptr tiles -> 8 banks.
        with tc.tile_pool(name="psS", bufs=2, space="PSUM") as psS, \
             tc.tile_pool(name="psB", bufs=2, space="PSUM") as psB, \
             tc.tile_pool(name="upool", bufs=4) as upool, \
             tc.tile_pool(name="opool", bufs=3) as opool, \
             tc.tile_pool(name="rcpool", bufs=4) as rcpool:

            def pv_half(m, us, o_nat, half):
                for ph in range(2):
                    if True:
                        h = 2 * m + ph
                        base = 64 * ph
                        u = us[ph]
                        pv = psB.tile([128, 4, DH + 1], F32, tag="pvtr",
                                      name="pv")
                        for q_ in range(4):
                            it = 4 * half + q_
                            for t in range(it + 1):
                                uo = U_OFF[t] + 128 * (it - t)
                                nc.tensor.matmul(pv[:, q_, :],
                                                 u[:, uo:uo + 128],
                                                 v_nat[:, t, :],
                                                 start=(t == 0), stop=False)
                            part = 32 * (h // 6)
                            noff = (h % 6) * N + it * 128
                            nc.tensor.matmul(
                                pv[:, q_, :],
                                unull_r4[part:part + 1, noff:noff + 128],
                                vnull16[part:part + 1, :],
                                start=False, stop=True)
                        rc = rcpool.tile([128, 4, 1], F32, tag="rc", name="rc")
                        nc.vector.reciprocal(out=rc[:], in_=pv[:, :, DH:DH + 1])
                        nc.vector.scalar_tensor_tensor(
                            out=o_nat[:, 4 * half:4 * half + 4,
                                      base:base + 64],
                            in0=pv[:, :, 0:DH], scalar=8.0,
                            in1=rc[:].broadcast_to([128, 4, DH]),
                            op0=ALU.mult, op1=ALU.mult)
            def pv_tail(m, o_nat, on_dve=False):
                # ptr allocated AFTER the 4 pv tiles so pv double-buffers
                ptr = psB.tile([128, NT, 128], BF16, tag="pvtr", name="ptr")
                for it in range(NT):
                    nc.tensor.transpose(ptr[:, it, :], o_nat[:, it, :],
                                        ident[:])
                # single fast DVE evac frees the PSUM bank; the fp8 hi/lo
                # split (x8 pre-scale, absorbed by LN2) runs on the idle
                # gpsimd from SBUF
                nc.vector.tensor_copy(outT[:, m, :, :], ptr[:])
                eng = nc.vector if on_dve else nc.gpsimd
                eng.tensor_copy(outT8h[:, m, :, :], outT[:, m, :, :])
                eng.tensor_sub(outT8l[:, m, :, :], outT[:, m, :, :],
                               outT8h[:, m, :, :])

            prev = None
            pending = None
            for m in range(NT):              # head pairs
                us = [upool.tile([128, U_COLS], BF16, tag="u", name="u")
                      for _ in range(2)]
                onat_prev = None
                if prev is not None:
                    onat_prev = opool.tile([128, NT, 128], BF16, tag="onat",
                                           name="o_nat")
                for gi, grp in enumerate(EXP_GROUPS):
                    goff = U_OFF[grp[0]]
                    for ph in range(2):
                        base = 64 * ph
                        u = us[ph]
                        ps = psS.tile([128, GW], F32, tag="scores", name="ps")
                        for t in grp:
                            lo = 128 * t
                            co = U_OFF[t] - goff
                            for c0, c1 in _bank_chunks(co, co + N - lo):
                                nc.tensor.matmul(
                                    ps[:, c0:c1],
                                    kT2[base:base + 64, lo:lo + 128],
                                    qT[base:base + 64, m,
                                       lo + (c0 - co):lo + (c1 - co)],
                                    start=True, stop=True)
                        nc.scalar.activation(out=u[:, goff:goff + GW],
                                             in_=ps[:], func=AF.Exp,
                                             scale=0.125)
                        for t in grp:  # causal band of each tile in group
                            nc.vector.tensor_mul(
                                u[:, U_OFF[t]:U_OFF[t] + 128],
                                u[:, U_OFF[t]:U_OFF[t] + 128], mtri[:])
                    # keep Act fed: spread the PV tail (of pair m-2), the
                    # q-projection and the prev-pair PV halves between the
                    # score groups so the post-G2 prelude is minimal
                    if gi == 0:
                        if pending is not None:
                            pv_tail(*pending)
                            pending = None
                        if m + 2 < NT:
                            q_pair(psB, m + 2)
                    if prev is not None and gi == 1:
                        pv_half(prev[0], prev[1], onat_prev, 0)
                    if prev is not None and gi == 2:
                        pv_half(prev[0], prev[1], onat_prev, 1)
                if m == 0:
                    # null rows to [h, i] + v^T -> natural [j, c] (PE work
                    # deferred off the phase-A critical path; inputs unullT /
                    # vts are long ready)
                    punT = psB.tile([H, NT, 128], BF16, tag="pvtr",
                                    name="punT")
                    for t in range(NT):
                        nc.tensor.transpose(punT[:, t, :], unullT[:, t, :],
                                            ident[:])
                    nc.vector.tensor_copy(unull[:], punT[:])
                    pvt = psB.tile([128, NT, DH], BF16, tag="pvtr", name="pvt")
                    for t in range(NT):
                        nc.tensor.transpose(pvt[:, t, :],
                                            vts[64:128, t * 128:(t + 1) * 128],
                                            ident[64:128, 64:128])
                    nc.vector.tensor_copy(v_nat[:, :, 0:DH], pvt[:])
                    # unull rows spread over partitions 0/32/64
                    for mm in range(NT):
                        part = 32 * ((2 * mm) // 6)
                        off = ((2 * mm) % 6) * N
                        nc.gpsimd.dma_start(
                            out=unull_r4[part:part + 1, off:off + 2 * N],
                            in_=unull[2 * mm:2 * mm + 2, :, :])
                    q_pair(psB, 1)
                if prev is not None:
                    pending = (prev[0], onat_prev)
                prev = (m, us)
            # flush: pair 6's tail (DVE - Pool would delay phase C), then
            # the final pair's halves + tail
            pv_tail(pending[0], pending[1], on_dve=True)
            onat = opool.tile([128, NT, 128], BF16, tag="onat", name="o_nat")
            pv_half(prev[0], prev[1], onat, 0)
            pv_half(prev[0], prev[1], onat, 1)
            pv_tail(prev[0], onat, on_dve=True)

        # ============ Phase C: out-projection + LN2 ============
        with tc.tile_pool(name="psC", bufs=8, space="PSUM") as psC:
            for it in range(NT):
                st = stage.tile([128, 2, 6], F32, tag="bnst")
                pos = []
                for ch in range(2):
                    cs = slice(ch * 512, (ch + 1) * 512)
                    po = psC.tile([128, 512], F32, tag="po")
                    i_ = 0
                    for tp in range(4):
                        for (a, b) in ((outT8h, woh), (outT8h, wol),
                                       (outT8l, woh)):
                            nc.tensor.matmul(
                                po[:], a[:, 2 * tp:2 * tp + 2, it, :],
                                b[:, 2 * tp:2 * tp + 2,
                                  ch * 512:(ch + 1) * 512],
                                start=(i_ == 0), stop=(i_ == 11),
                                perf_mode=DR)
                            i_ += 1
                    nc.vector.bn_stats(out=st[:, ch, :], in_=po[:])
                    pos.append(po)
                mv = stage.tile([128, 2], F32, tag="bnmv")
                nc.vector.bn_aggr(out=mv[:], in_=st[:])
                rstd = stage.tile([128, 1], F32, tag="rstd")
                nc.scalar.activation(out=rstd[:], in_=mv[:, 1:2], func=AF.Sqrt,
                                     bias=eps_t[:], scale=1.0)
                nc.vector.reciprocal(out=rstd[:], in_=rstd[:])
                negmr = stage.tile([128, 1], F32, tag="negmr")
                nc.vector.tensor_scalar(out=negmr[:], in0=mv[:, 0:1],
                                        scalar1=rstd[:], scalar2=-1.0,
                                        op0=ALU.mult, op1=ALU.mult)
                o_s = stage.tile([128, D], F32, tag="os", bufs=4)
                for ch in range(2):
                    cs = slice(ch * 512, (ch + 1) * 512)
                    # LN2 apply on the (C-phase idle) Act engine:
                    # o = po*rstd - mean*rstd
                    nc.scalar.activation(out=o_s[:, cs], in_=pos[ch][:],
                                         func=AF.Identity, bias=negmr[:],
                                         scale=rstd[:])
                    if apply_gout:
                        nc.gpsimd.tensor_mul(o_s[:, cs], o_s[:, cs],
                                             gout_b[:, cs])
                    nc.sync.dma_start(out=out_d[it * 128:(it + 1) * 128, cs],
                                      in_=o_s[:, cs])


_CACHED = {}


def _get_nc(apply_gout=False):
    if apply_gout not in _CACHED:
        nc = bacc.Bacc("TRN2", target_bir_lowering=False, debug=False)
        _emit(nc, apply_gout)
        nc.compile()
        _CACHED[apply_gout] = nc
    return _CACHED[apply_gout]


def make_in_maps(x, mask, g_in, Wq, Wkv, null_kv, Wo, g_out):
    b = x.shape[0]
    BF = ml_dtypes.bfloat16
    E4 = ml_dtypes.float8_e4m3
    g = g_in.astype(np.float64)
    W2 = Wq.astype(np.float64) * g[None, :]                  # [INNER, D]
    Wqq = W2 - W2.sum(axis=1, keepdims=True) / D             # fold mean removal
    wn = np.einsum('k,hkd->hd', null_kv[0].astype(np.float64),
                   Wqq.reshape(H, DH, D))                    # [H, D]
    r = (1.0 / np.sqrt(x.astype(np.float64).var(axis=-1) + EPS)) / (SW * SX)
    xT = np.transpose(x.astype(np.float64), (0, 2, 1)) * SX  # [b, D, N]
    mask_u8 = np.ascontiguousarray(mask).view(np.uint8) if mask.dtype == np.bool_ \
        else mask.astype(np.uint8)

    def pack(a):
        # [D, cols] -> partition-major [128, NT, cols] (1 descriptor/partition)
        cols = a.shape[1]
        return np.ascontiguousarray(
            a.reshape(NT, 128, cols).transpose(1, 0, 2))

    def split8(a):
        hi = a.astype(E4)
        lo = (a - hi.astype(np.float64)).astype(E4)
        return pack(hi), pack(lo)

    wq_pm = (Wqq.T * SW).reshape(NT, 128, NT, DH * 2).transpose(1, 2, 0, 3)
    wqh = np.ascontiguousarray(wq_pm.astype(E4))
    wql = np.ascontiguousarray((wq_pm - wqh.astype(np.float64)).astype(E4))
    wkvh, wkvl = split8(Wkv.astype(np.float64).T * SW)
    wnh, wnl = split8(wn.T * SW)
    woh, wol = split8(Wo.astype(np.float64).T * SW)
    shared = {
        "wqh": wqh, "wql": wql, "wkvh": wkvh, "wkvl": wkvl,
        "wnh": wnh, "wnl": wnl, "woh": woh, "wol": wol,
        "vnull": np.ascontiguousarray(null_kv[1].astype(BF)),
        "gout": np.ascontiguousarray(g_out.astype(np.float32)),
    }
    maps = []
    for c in range(b):
        xch, xcl = split8(xT[c])
        maps.append({"xh": xch, "xl": xcl, "mask": mask_u8[c],
                     "rrow": np.ascontiguousarray(r[c].astype(np.float32)),
                     **shared})
    return maps


def kernel(x, mask, g_in, Wq, Wkv, null_kv, Wo, g_out):
    x = np.asarray(x)
    mask = np.asarray(mask)
    g_in, g_out = np.asarray(g_in), np.asarray(g_out)
    Wq, Wkv, Wo = np.asarray(Wq), np.asarray(Wkv), np.asarray(Wo)
    null_kv = np.asarray(null_kv)
    b = x.shape[0]
    assert x.shape == (b, N, D) and b == 8
    in_maps = make_in_maps(x, mask, g_in, Wq, Wkv, null_kv, Wo, g_out)
    nc = _get_nc(apply_gout=not bool(np.all(g_out == 1.0)))
    res = run_bass_kernel_spmd(nc, in_maps, core_ids=list(range(b)))
    return np.stack([res.results[c]["out"] for c in range(b)], axis=0)


# revision 29
# speedup vs baseline: 1.0023x; 1.0023x over previous
"""Trainium2 Bass kernel for the masked MQA attention block (nn_Attention_4252017623134).

Sharding: pure data-parallel over batch. b=8 batch elements, 8 NeuronCores,
one batch element per core, weights replicated. No collectives.

Per-core math (n=1024, d=1024, h=16, dh=64, inner=1024):
  context = x                      (pre-norm residual branch feeds K/V)
  xn  = layernorm(x) * g_in
  q   = xn @ Wq.T   (per head, scaled by 1/8 = dh^-0.5, folded into exp scale)
  k,v = context @ Wkv.T (single shared KV head) + null_kv token
  att = softmax(mask(q k^T / 8))   (padding + causal(key j visible iff j <= i))
  out = layernorm(att @ v @ Wo.T) * g_out

Key design decisions (v2 — fp8 DoubleRow projections):
  * All deep-K projections (q, kv, null-score) run as fp8e4m3 DoubleRow
    matmuls with an error-compensated hi/lo split: A@B ~ Ah@Bh + Ah@Bl +
    Al@Bh where Ah=fp8(A), Al=fp8(A-Ah). DoubleRow contracts 256 rows per
    instruction at 0.5 cycles/output-column, so the 3-term split costs
    0.75x of bf16 while injecting only ~1.2e-3 relative error (validated
    on HW). x is pre-scaled by 8 and the weights by 32 host-side so the
    fp8 residuals stay in e4m3's normal range; the 1/256 compensations
    fold into the host-computed rstd row (q/null paths) and the kv-evac
    scalar multiply.
  * LN1 folded into the q-projection: q_i = r_i * (Wq'' @ x_i) with
    Wq'' = Wq*diag(g) - outer(Wq@g, 1)/D precomputed on HOST and
    r_i = rsqrt(var_i+eps)/256 shipped as a row like the baseline.
  * Null-token scores via a TRANSPOSED projection: pnullT[i, h] accumulated
    with x as the stationary operand (16 moving columns -> ~0.8k PE cycles
    instead of 8k), exp'd as one [128, 8x16] activation, PE-transposed to
    row form, and spread over partitions 0/32/64/96 (unull_r4) so the PV
    null matmuls can address any head from a legal lhsT base partition.
  * Padding mask applied by ZEROING masked k/v columns; masked j gives
    u=exp(0)=1 but contributes v_j=0 and is excluded from the denominator
    via a mask column appended to V.
  * Scores stay bf16 (fp8 q/k injects ~5e-2 error - measured), computed
    transposed (simT[j,i], exact visible windows). Exp groups pack the 8
    j-tile windows into exactly 3x1536 columns ({0,4},{1,3},{2,5,6,7}),
    each a single [128,1536] 3-bank PSUM tile -> 3 activation instructions
    per head instead of 5 (the Act engine is the #2 engine at ~75us).
    Q-projection moved wholly to phase A to free the PSUM banks for this.
  * P@V runs NATURAL: lhsT = u[j, i-chunk], rhs = v_nat [j, 64ch + mask
    col]; softmax denominator lands in PSUM col 64; division fused into
    the PV evac as a per-partition reciprocal multiply. PV output returns
    to [c, i] via bf16 PE transposes staged through a 1-bank PSUM tile
    shared with the pv pool (ptr allocated after the 4th pv tile so pv
    stays double-buffered).
  * Engine choreography: input DMAs split across SP, Act-HWDGE and
    gpsimd-SWDGE queues (xh per-tile first so the kv projection starts
    ~1.5us in); causal-band zeroing (mtri) on DVE; LN2 gain multiply on
    gpsimd; dependency-free identity matmuls warm the PE p-state.
"""

import contextlib

import numpy as np
import ml_dtypes

import concourse.bass as bass
import concourse.bacc as bacc
import concourse.tile as tile
import concourse.mybir as mybir
from concourse.bass_utils import run_bass_kernel_spmd
from concourse.masks import make_identity

N = 1024          # sequence length per core
D = 1024          # model dim
H = 16            # query heads
DH = 64           # head dim
INNER = H * DH    # 1024
NT = N // 128     # 8 i-tiles / j-tiles / d-tiles
EPS = 1e-5
SW = 32.0         # host pre-scale on weights (fp8 range centering)
SX = 8.0          # host pre-scale on x
SINV = 1.0 / (SW * SX)

F32 = mybir.dt.float32
BF16 = mybir.dt.bfloat16
F8 = mybir.dt.float8e4
U8 = mybir.dt.uint8
AF = mybir.ActivationFunctionType
ALU = mybir.AluOpType
DR = mybir.MatmulPerfMode.DoubleRow

# exp groups per head: j-tile windows (N - 128t) packed into three
# [128, 1536] fp32 (3-bank) PSUM tiles; 1024+512, 896+640, 768+384+256+128.
EXP_GROUPS = [(0, 4), (1, 3), (2, 5, 6, 7)]
GW = 1536
U_OFF = {}
_off = 0
for _g in EXP_GROUPS:
    for _t in _g:
        U_OFF[_t] = _off
        _off += N - 128 * _t
U_COLS = _off  # 4608


def _bank_chunks(c0, c1):
    """Split [c0, c1) at 512-column (2KB fp32 PSUM bank) boundaries."""
    out = []
    while c0 < c1:
        nxt = min(c1, (c0 // 512 + 1) * 512)
        out.append((c0, nxt))
        c0 = nxt
    return out


def _emit(nc, apply_gout=True):
    # ---------------- DRAM I/O ----------------
    # all big operands ship pre-packed [128, NT, cols] (partition-major,
    # one contiguous descriptor per partition)
    xh_d = nc.dram_tensor("xh", [128, NT, N], F8, kind="ExternalInput")
    xl_d = nc.dram_tensor("xl", [128, NT, N], F8, kind="ExternalInput")
    # wq is packed PAIR-major: [128, pair, t, 128] so per-pair slices are one
    # contiguous descriptor per partition
    wqh_d = nc.dram_tensor("wqh", [128, NT, NT, 128], F8, kind="ExternalInput")
    wql_d = nc.dram_tensor("wql", [128, NT, NT, 128], F8, kind="ExternalInput")
    wkvh_d = nc.dram_tensor("wkvh", [128, NT, 2 * DH], F8, kind="ExternalInput")
    wkvl_d = nc.dram_tensor("wkvl", [128, NT, 2 * DH], F8, kind="ExternalInput")
    wnh_d = nc.dram_tensor("wnh", [128, NT, H], F8, kind="ExternalInput")
    wnl_d = nc.dram_tensor("wnl", [128, NT, H], F8, kind="ExternalInput")
    woh_d = nc.dram_tensor("woh", [128, NT, D], F8, kind="ExternalInput")
    wol_d = nc.dram_tensor("wol", [128, NT, D], F8, kind="ExternalInput")
    vnull_d = nc.dram_tensor("vnull", [DH], BF16, kind="ExternalInput")
    mask_d = nc.dram_tensor("mask", [N], U8, kind="ExternalInput")
    rrow_d = nc.dram_tensor("rrow", [N], F32, kind="ExternalInput")
    gout_d = nc.dram_tensor("gout", [D], F32, kind="ExternalInput")
    out_d = nc.dram_tensor("out", [N, D], F32, kind="ExternalOutput")

    d_ = dict(xh_d=xh_d, xl_d=xl_d, wqh_d=wqh_d, wql_d=wql_d,
              wkvh_d=wkvh_d, wkvl_d=wkvl_d, wnh_d=wnh_d, wnl_d=wnl_d,
              woh_d=woh_d, wol_d=wol_d, vnull_d=vnull_d, mask_d=mask_d,
              rrow_d=rrow_d,
              gout_d=gout_d, out_d=out_d)
    with tile.TileContext(nc) as tc:
        _emit_tile(nc, tc, d_, apply_gout)
    return nc


def _emit_tile(nc, tc, d_, apply_gout):
    xh_d, xl_d = d_["xh_d"], d_["xl_d"]
    wqh_d, wql_d = d_["wqh_d"], d_["wql_d"]
    wkvh_d, wkvl_d = d_["wkvh_d"], d_["wkvl_d"]
    wnh_d, wnl_d = d_["wnh_d"], d_["wnl_d"]
    woh_d, wol_d = d_["woh_d"], d_["wol_d"]
    vnull_d, mask_d = d_["vnull_d"], d_["mask_d"]
    rrow_d, gout_d, out_d = d_["rrow_d"], d_["gout_d"], d_["out_d"]

    ctx = contextlib.ExitStack()
    with ctx:
        consts = ctx.enter_context(tc.tile_pool(name="consts", bufs=1))
        persist = ctx.enter_context(tc.tile_pool(name="persist", bufs=1))
        stage = ctx.enter_context(tc.tile_pool(name="stage", bufs=6))

        # ---------------- constants (no DMA deps: emitted first so the PE
        # warm-up and gpsimd run from t=0) ----------------
        ident = consts.tile([128, 128], BF16)
        make_identity(nc, ident[:])
        # causal 0/1 band mask: keep u[j_rel, i_rel] iff i_rel >= j_rel
        mtri = consts.tile([128, 128], BF16)
        nc.gpsimd.memset(mtri[:], 1.0)
        nc.gpsimd.affine_select(out=mtri[:], in_=mtri[:], compare_op=ALU.is_ge,
                                fill=0.0, base=0, pattern=[[1, 128]],
                                channel_multiplier=-1)
        eps_t = consts.tile([128, 1], F32)
        nc.vector.memset(eps_t[:], EPS)

        # ------------- persistent tiles -------------
        kT2 = persist.tile([128, N], BF16, tag="kT2")       # k^T in both halves
        vts = persist.tile([128, N], BF16, tag="vts")   # rows 64:128 = masked v^T
        v_nat = persist.tile([128, NT, DH + 1], BF16, tag="v_nat")  # col 64 = mask
        vnull16 = persist.tile([128, DH + 1], BF16, tag="vnull16")  # rows 0/32/64
        unull = persist.tile([H, NT, 128], BF16, tag="unull")
        unull_r4 = persist.tile([128, 6 * N], BF16, tag="unull_r4")  # rows 0/32/64
        unullT = persist.tile([128, NT, H], BF16, tag="unullT")
        outT = persist.tile([128, NT, NT, 128], BF16, tag="outT")  # [c,pair,it,i]
        outT8h = persist.tile([128, NT, NT, 128], F8, tag="outT8h")
        outT8l = persist.tile([128, NT, NT, 128], F8, tag="outT8l")
        rbroad = persist.tile([128, N], F32, tag="rbroad")  # rstd/256 broadcast
        rcolT = persist.tile([128, NT], F32, tag="rcolT")   # rstd/256 column form
        qT = persist.tile([128, NT, N], BF16, tag="qT")     # q^T (pair slabs)
        xh = persist.tile([128, NT, N], F8, tag="xh")       # x^T hi (x8)
        xl = persist.tile([128, NT, N], F8, tag="xl")       # x^T lo residual
        wqh = persist.tile([128, NT, NT, 128], F8, tag="wqh")  # [p, pair, t, ch]
        wql = persist.tile([128, NT, NT, 128], F8, tag="wql")
        wkvh = persist.tile([128, NT, 2 * DH], F8, tag="wkvh")
        wkvl = persist.tile([128, NT, 2 * DH], F8, tag="wkvl")
        wnh = persist.tile([128, NT, H], F8, tag="wnh")
        wnl = persist.tile([128, NT, H], F8, tag="wnl")
        woh = persist.tile([128, NT, D], F8, tag="woh")
        wol = persist.tile([128, NT, D], F8, tag="wol")
        gout_b = persist.tile([128, D], F32, tag="gout_b")
        maskb_u8 = consts.tile([128, N], U8)
        maskc_u8 = consts.tile([128, NT], U8)
        maskb = consts.tile([128, N], BF16)
        maskc = consts.tile([128, NT], BF16)

        # ---- DMA issues. The DMA transfers serialize on ONE channel
        # (~0.385ns per per-partition byte), so channel ORDER sets the
        # phase-B start: kv/q0 inputs first, masks cheap-early, big late
        # tensors (wo, gout) last. Act queue kept to 5 issues (SEQ cost). ----
        nc.sync.dma_start(out=maskb_u8[:],
                          in_=bass.AP(tensor=mask_d, offset=0,
                                      ap=[[0, 128], [1, N]]))
        nc.scalar.dma_start(out=wkvh[:], in_=wkvh_d.ap())
        nc.sync.dma_start(out=xh[:, 0:4, :], in_=xh_d[:, 0:4, :])
        nc.scalar.dma_start(out=wkvl[:], in_=wkvl_d.ap())
        nc.sync.dma_start(out=xh[:, 4:8, :], in_=xh_d[:, 4:8, :])
        nc.scalar.dma_start(out=wqh[:, 0, :, :], in_=wqh_d[:, 0, :, :])
        nc.sync.dma_start(out=wql[:, 0, :, :], in_=wql_d[:, 0, :, :])
        nc.scalar.dma_start(out=wqh[:, 1, :, :], in_=wqh_d[:, 1, :, :])
        nc.sync.dma_start(out=rbroad[:],
                          in_=bass.AP(tensor=rrow_d, offset=0,
                                      ap=[[0, 128], [1, N]]))
        nc.scalar.dma_start(out=maskc_u8[:],
                            in_=bass.AP(tensor=mask_d, offset=0,
                                        ap=[[1, 128], [128, NT]]))
        nc.sync.dma_start(out=wql[:, 1, :, :], in_=wql_d[:, 1, :, :])
        nc.sync.dma_start(out=wnh[:], in_=wnh_d.ap())
        nc.sync.dma_start(out=wnl[:], in_=wnl_d.ap())
        nc.sync.dma_start(out=rcolT[:],
                          in_=bass.AP(tensor=rrow_d, offset=0,
                                      ap=[[1, 128], [128, NT]]))
        for m in range(2, NT):
            nc.sync.dma_start(out=wqh[:, m, :, :], in_=wqh_d[:, m, :, :])
            nc.sync.dma_start(out=wql[:, m, :, :], in_=wql_d[:, m, :, :])
        for r in (0, 32, 64):
            nc.sync.dma_start(out=vnull16[r:r + 1, 0:DH],
                              in_=bass.AP(tensor=vnull_d, offset=0,
                                          ap=[[0, 1], [1, DH]]))
        nc.sync.dma_start(out=gout_b[:],
                          in_=bass.AP(tensor=gout_d, offset=0,
                                      ap=[[0, 128], [1, D]]))
        nc.sync.dma_start(out=woh[:], in_=woh_d.ap())
        nc.sync.dma_start(out=wol[:], in_=wol_d.ap())

        # xl SWDGE issues first        # xl SWDGE issues first        # xl SWDGE issues first        # xl SWDGE issues first        # xl SWDGE issues first        # xl SWDGE issues first (no deps), then the mask converts (which
        # wait on the maskb DMA)
        nc.gpsimd.dma_start(out=xl[:, 0:4, :], in_=xl_d[:, 0:4, :])
        nc.gpsimd.dma_start(out=xl[:, 4:8, :], in_=xl_d[:, 4:8, :])
        nc.gpsimd.tensor_copy(maskb[:], maskb_u8[:])
        nc.gpsimd.tensor_copy(maskc[:], maskc_u8[:])
        nc.gpsimd.memset(vnull16[:, DH:DH + 1], 1.0)

        def q_pair(pool, m, tag="pvtr"):
            for ch in range(2):
                cs = slice(ch * 512, (ch + 1) * 512)
                pq = pool.tile([128, 512], F32, tag=tag, name="pq")
                i_ = 0
                for (a, b, tp) in [(wqh, xh, 0), (wqh, xh, 1),
                                   (wqh, xh, 2), (wqh, xh, 3),
                                   (wqh, xl, 0), (wqh, xl, 1),
                                   (wqh, xl, 2), (wqh, xl, 3),
                                   (wql, xh, 0), (wql, xh, 1),
                                   (wql, xh, 2), (wql, xh, 3)]:
                    nc.tensor.matmul(pq[:],
                                     a[:, m, 2 * tp:2 * tp + 2, :],
                                     b[:, 2 * tp:2 * tp + 2, cs],
                                     start=(i_ == 0), stop=(i_ == 11),
                                     perf_mode=DR)
                    i_ += 1
                nc.vector.scalar_tensor_tensor(
                    out=qT[:, m, cs], in0=pq[:], scalar=1.0,
                    in1=rbroad[:, cs], op0=ALU.mult, op1=ALU.mult)

        # ====== Phase A: projections (fp8 DoubleRow hi/lo), null scores ======
        with tc.tile_pool(name="psA", bufs=1, space="PSUM") as psA, \
             tc.tile_pool(name="psQ", bufs=2, space="PSUM") as psQp:
            pkv = psA.tile([128, N], F32, tag="pkv")
            pwarm = psA.tile([128, 128], F32, tag="pwarm")

            def warm(n):
                # PE p-state bridge: dependency-free matmuls fill DMA-wait
                # gaps so the ramp (2.4GHz after 3us continuous) survives
                for w_ in range(n):
                    nc.tensor.matmul(pwarm[:], ident[:], ident[:],
                                     start=(w_ == 0), stop=(w_ == n - 1))

            warm(34)

            # --- kv projection: 3-term hi/lo DoubleRow (tp order follows
            #     the split xh/xl DMA halves) ---
            for ch in range(2):
                cs = slice(ch * 512, (ch + 1) * 512)
                i_ = 0
                for (a, b, tp) in [(wkvh, xh, 0), (wkvh, xh, 1),
                                   (wkvl, xh, 0), (wkvl, xh, 1),
                                   (wkvh, xl, 0), (wkvh, xl, 1),
                                   (wkvh, xh, 2), (wkvh, xh, 3),
                                   (wkvl, xh, 2), (wkvl, xh, 3),
                                   (wkvh, xl, 2), (wkvh, xl, 3)]:
                    nc.tensor.matmul(pkv[:, cs], a[:, 2 * tp:2 * tp + 2, :],
                                     b[:, 2 * tp:2 * tp + 2, cs],
                                     start=(i_ == 0), stop=(i_ == 11),
                                     perf_mode=DR)
                    i_ += 1
            # evac: masked k^T (low half), masked v^T (rows 64:128)
            nc.vector.scalar_tensor_tensor(out=kT2[0:64, :], in0=pkv[0:64, :],
                                           scalar=SINV, in1=maskb[0:64, :],
                                           op0=ALU.mult, op1=ALU.mult)
            # k^T duplicate into partitions 64:128 for parity-B score matmuls
            nc.gpsimd.dma_start(out=kT2[64:128, :], in_=kT2[0:64, :])

            # --- q projection: 3-term hi/lo DoubleRow, LN1 via rstd row ---
            warm(14)
            q_pair(psQp, 0, tag="pq")
            # v evac after the q0 evacs (v is needed ~5us later than qT0)
            nc.vector.scalar_tensor_tensor(out=vts[64:128, :],
                                           in0=pkv[64:128, :],
                                           scalar=SINV, in1=maskb[64:128, :],
                                           op0=ALU.mult, op1=ALU.mult)

            # --- null-score projection, TRANSPOSED (x stationary) ---
            warm(12)
            pnullT = psA.tile([128, NT, H], F32, tag="pnullT")
            for it in range(NT):
                i_ = 0
                for (a, b) in ((xh, wnh), (xh, wnl), (xl, wnh)):
                    for tp in range(4):
                        nc.tensor.matmul(pnullT[:, it, :],
                                         a[:, 2 * tp:2 * tp + 2,
                                           it * 128:(it + 1) * 128],
                                         b[:, 2 * tp:2 * tp + 2, :],
                                         start=(i_ == 0), stop=(i_ == 11),
                                         perf_mode=DR)
                        i_ += 1
            # null exp straight from PSUM with the per-i rstd/2048 scale
            rcolT8 = stage.tile([128, NT], F32, tag="rcol8", bufs=1)
            nc.vector.tensor_scalar(out=rcolT8[:], in0=rcolT[:], scalar1=0.125,
                                    scalar2=None, op0=ALU.mult)
            for t in range(NT):
                nc.scalar.activation(out=unullT[:, t, :], in_=pnullT[:, t, :],
                                     func=AF.Exp, scale=rcolT8[:, t:t + 1])
            for t in range(NT):
                nc.gpsimd.tensor_copy(v_nat[:, t, DH:DH + 1], maskc[:, t:t + 1])
            # (q1 and the null/v transposes run inside phase B's first pair,
            #  off the phase-A PE critical path)

        # ============ Phase B: attention, per head pair ============
        # psS: 2 x [128,1536] (3 banks each); psB: 2 x 1 bank shared by the
        # q-projection pq, PV pv, and transpose-staging ptr tiles -> 8 banks.
        with tc.tile_pool(name="psS", bufs=2, space="PSUM") as psS, \
             tc.tile_pool(name="psB", bufs=2, space="PSUM") as psB, \
             tc.tile_pool(name="upool", bufs=4) as upool, \
             tc.tile_pool(name="opool", bufs=3) as opool, \
             tc.tile_pool(name="rcpool", bufs=4) as rcpool:

            def pv_half(m, us, o_nat, half):
                for ph in range(2):
                    if True:
                        h = 2 * m + ph
                        base = 64 * ph
                        u = us[ph]
                        pv = psB.tile([128, 4, DH + 1], F32, tag="pvtr",
                                      name="pv")
                        for q_ in range(4):
                            it = 4 * half + q_
                            for t in range(it + 1):
                                uo = U_OFF[t] + 128 * (it - t)
                                nc.tensor.matmul(pv[:, q_, :],
                                                 u[:, uo:uo + 128],
                                                 v_nat[:, t, :],
                                                 start=(t == 0), stop=False)
                            part = 32 * (h // 6)
                            noff = (h % 6) * N + it * 128
                            nc.tensor.matmul(
                                pv[:, q_, :],
                                unull_r4[part:part + 1, noff:noff + 128],
                                vnull16[part:part + 1, :],
                                start=False, stop=True)
                        rc = rcpool.tile([128, 4, 1], F32, tag="rc", name="rc")
                        nc.vector.reciprocal(out=rc[:], in_=pv[:, :, DH:DH + 1])
                        nc.vector.scalar_tensor_tensor(
                            out=o_nat[:, 4 * half:4 * half + 4,
                                      base:base + 64],
                            in0=pv[:, :, 0:DH], scalar=8.0,
                            in1=rc[:].broadcast_to([128, 4, DH]),
                            op0=ALU.mult, op1=ALU.mult)
            def pv_tail(m, o_nat, on_dve=False):
                # ptr allocated AFTER the 4 pv tiles so pv double-buffers
                ptr = psB.tile([128, NT, 128], BF16, tag="pvtr", name="ptr")
                for it in range(NT):
                    nc.tensor.transpose(ptr[:, it, :], o_nat[:, it, :],
                                        ident[:])
                # single fast DVE evac frees the PSUM bank; the fp8 hi/lo
                # split (x8 pre-scale, absorbed by LN2) runs on the idle
                # gpsimd from SBUF
                nc.vector.tensor_copy(outT[:, m, :, :], ptr[:])
                eng = nc.vector if on_dve else nc.gpsimd
                eng.tensor_copy(outT8h[:, m, :, :], outT[:, m, :, :])
                eng.tensor_sub(outT8l[:, m, :, :], outT[:, m, :, :],
                               outT8h[:, m, :, :])

            prev = None
            for m in range(NT):              # head pairs
                us = [upool.tile([128, U_COLS], BF16, tag="u", name="u")
                      for _ in range(2)]
                onat_prev = None
                if prev is not None:
                    onat_prev = opool.tile([128, NT, 128], BF16, tag="onat",
                                           name="o_nat")
                for gi, grp in enumerate(EXP_GROUPS):
                    goff = U_OFF[grp[0]]
                    for ph in range(2):
                        base = 64 * ph
                        u = us[ph]
                        ps = psS.tile([128, GW], F32, tag="scores", name="ps")
                        for t in grp:
                            lo = 128 * t
                            co = U_OFF[t] - goff
                            for c0, c1 in _bank_chunks(co, co + N - lo):
                                nc.tensor.matmul(
                                    ps[:, c0:c1],
                                    kT2[base:base + 64, lo:lo + 128],
                                    qT[base:base + 64, m,
                                       lo + (c0 - co):lo + (c1 - co)],
                                    start=True, stop=True)
                        nc.scalar.activation(out=u[:, goff:goff + GW],
                                             in_=ps[:], func=AF.Exp,
                                             scale=0.125)
                        for t in grp:  # causal band of each tile in group
                            nc.vector.tensor_mul(
                                u[:, U_OFF[t]:U_OFF[t] + 128],
                                u[:, U_OFF[t]:U_OFF[t] + 128], mtri[:])
                    # keep Act fed: spread the q-projection and prev-pair PV
                    # between the score groups so the post-G2 prelude to the
                    # next pair's scores is short
                    if gi == 0 and m + 2 < NT:
                        q_pair(psB, m + 2)
                    if prev is not None and gi == 1:
                        pv_half(prev[0], prev[1], onat_prev, 0)
                    if prev is not None and gi == 2:
                        pv_half(prev[0], prev[1], onat_prev, 1)
                if m == 0:
                    # null rows to [h, i] + v^T -> natural [j, c] (PE work
                    # deferred off the phase-A critical path; inputs unullT /
                    # vts are long ready)
                    punT = psB.tile([H, NT, 128], BF16, tag="pvtr",
                                    name="punT")
                    for t in range(NT):
                        nc.tensor.transpose(punT[:, t, :], unullT[:, t, :],
                                            ident[:])
                    nc.vector.tensor_copy(unull[:], punT[:])
                    pvt = psB.tile([128, NT, DH], BF16, tag="pvtr", name="pvt")
                    for t in range(NT):
                        nc.tensor.transpose(pvt[:, t, :],
                                            vts[64:128, t * 128:(t + 1) * 128],
                                            ident[64:128, 64:128])
                    nc.vector.tensor_copy(v_nat[:, :, 0:DH], pvt[:])
                    # unull rows spread over partitions 0/32/64
                    for mm in range(NT):
                        part = 32 * ((2 * mm) // 6)
                        off = ((2 * mm) % 6) * N
                        nc.gpsimd.dma_start(
                            out=unull_r4[part:part + 1, off:off + 2 * N],
                            in_=unull[2 * mm:2 * mm + 2, :, :])
                    q_pair(psB, 1)
                if prev is not None:
                    pv_tail(prev[0], onat_prev)
                prev = (m, us)
            onat = opool.tile([128, NT, 128], BF16, tag="onat", name="o_nat")
            pv_half(prev[0], prev[1], onat, 0)
            pv_half(prev[0], prev[1], onat, 1)
            pv_tail(prev[0], onat, on_dve=True)

        # ============ Phase C: out-projection + LN2 ============
        with tc.tile_pool(name="psC", bufs=8, space="PSUM") as psC:
            for it in range(NT):
                st = stage.tile([128, 2, 6], F32, tag="bnst")
                pos = []
                for ch in range(2):
                    cs = slice(ch * 512, (ch + 1) * 512)
                    po = psC.tile([128, 512], F32, tag="po")
                    i_ = 0
                    for tp in range(4):
                        for (a, b) in ((outT8h, woh), (outT8h, wol),
                                       (outT8l, woh)):
                            nc.tensor.matmul(
                                po[:], a[:, 2 * tp:2 * tp + 2, it, :],
                                b[:, 2 * tp:2 * tp + 2,
                                  ch * 512:(ch + 1) * 512],
                                start=(i_ == 0), stop=(i_ == 11),
                                perf_mode=DR)
                            i_ += 1
                    nc.vector.bn_stats(out=st[:, ch, :], in_=po[:])
                    pos.append(po)
                mv = stage.tile([128, 2], F32, tag="bnmv")
                nc.vector.bn_aggr(out=mv[:], in_=st[:])
                rstd = stage.tile([128, 1], F32, tag="rstd")
                nc.scalar.activation(out=rstd[:], in_=mv[:, 1:2], func=AF.Sqrt,
                                     bias=eps_t[:], scale=1.0)
                nc.vector.reciprocal(out=rstd[:], in_=rstd[:])
                negmr = stage.tile([128, 1], F32, tag="negmr")
                nc.vector.tensor_scalar(out=negmr[:], in0=mv[:, 0:1],
                                        scalar1=rstd[:], scalar2=-1.0,
                                        op0=ALU.mult, op1=ALU.mult)
                o_s = stage.tile([128, D], F32, tag="os", bufs=4)
                for ch in range(2):
                    cs = slice(ch * 512, (ch + 1) * 512)
                    # LN2 apply on the (C-phase idle) Act engine:
                    # o = po*rstd - mean*rstd
                    nc.scalar.activation(out=o_s[:, cs], in_=pos[ch][:],
                                         func=AF.Identity, bias=negmr[:],
                                         scale=rstd[:])
                    if apply_gout:
                        nc.gpsimd.tensor_mul(o_s[:, cs], o_s[:, cs],
                                             gout_b[:, cs])
                    nc.sync.dma_start(out=out_d[it * 128:(it + 1) * 128, cs],
                                      in_=o_s[:, cs])


_CACHED = {}


def _get_nc(apply_gout=False):
    if apply_gout not in _CACHED:
        nc = bacc.Bacc("TRN2", target_bir_lowering=False, debug=False)
        _emit(nc, apply_gout)
        nc.compile()
        _CACHED[apply_gout] = nc
    return _CACHED[apply_gout]


def make_in_maps(x, mask, g_in, Wq, Wkv, null_kv, Wo, g_out):
    b = x.shape[0]
    BF = ml_dtypes.bfloat16
    E4 = ml_dtypes.float8_e4m3
    g = g_in.astype(np.float64)
    W2 = Wq.astype(np.float64) * g[None, :]                  # [INNER, D]
    Wqq = W2 - W2.sum(axis=1, keepdims=True) / D             # fold mean removal
    wn = np.einsum('k,hkd->hd', null_kv[0].astype(np.float64),
                   Wqq.reshape(H, DH, D))                    # [H, D]
    r = (1.0 / np.sqrt(x.astype(np.float64).var(axis=-1) + EPS)) / (SW * SX)
    xT = np.transpose(x.astype(np.float64), (0, 2, 1)) * SX  # [b, D, N]
    mask_u8 = np.ascontiguousarray(mask).view(np.uint8) if mask.dtype == np.bool_ \
        else mask.astype(np.uint8)

    def pack(a):
        # [D, cols] -> partition-major [128, NT, cols] (1 descriptor/partition)
        cols = a.shape[1]
        return np.ascontiguousarray(
            a.reshape(NT, 128, cols).transpose(1, 0, 2))

    def split8(a):
        hi = a.astype(E4)
        lo = (a - hi.astype(np.float64)).astype(E4)
        return pack(hi), pack(lo)

    wq_pm = (Wqq.T * SW).reshape(NT, 128, NT, DH * 2).transpose(1, 2, 0, 3)
    wqh = np.ascontiguousarray(wq_pm.astype(E4))
    wql = np.ascontiguousarray((wq_pm - wqh.astype(np.float64)).astype(E4))
    wkvh, wkvl = split8(Wkv.astype(np.float64).T * SW)
    wnh, wnl = split8(wn.T * SW)
    woh, wol = split8(Wo.astype(np.float64).T * SW)
    shared = {
        "wqh": wqh, "wql": wql, "wkvh": wkvh, "wkvl": wkvl,
        "wnh": wnh, "wnl": wnl, "woh": woh, "wol": wol,
        "vnull": np.ascontiguousarray(null_kv[1].astype(BF)),
        "gout": np.ascontiguousarray(g_out.astype(np.float32)),
    }
    maps = []
    for c in range(b):
        xch, xcl = split8(xT[c])
        maps.append({"xh": xch, "xl": xcl, "mask": mask_u8[c],
                     "rrow": np.ascontiguousarray(r[c].astype(np.float32)),
                     **shared})
    return maps


def kernel(x, mask, g_in, Wq, Wkv, null_kv, Wo, g_out):
    x = np.asarray(x)
    mask = np.asarray(mask)
    g_in, g_out = np.asarray(g_in), np.asarray(g_out)
    Wq, Wkv, Wo = np.asarray(Wq), np.asarray(Wkv), np.asarray(Wo)
    null_kv = np.asarray(null_kv)
    b = x.shape[0]
    assert x.shape == (b, N, D) and b == 8
    in_maps = make_in_maps(x, mask, g_in, Wq, Wkv, null_kv, Wo, g_out)
    nc = _get_nc(apply_gout=not bool(np.all(g_out == 1.0)))
    res = run_bass_kernel_spmd(nc, in_maps, core_ids=list(range(b)))
    return np.stack([res.results[c]["out"] for c in range(b)], axis=0)
